# revision 8
# baseline (speedup 1.0000x reference)
"""ARD RBF kernel matrix on 8 TRN2 NeuronCores.

out[n, m] = exp(log_outputscale) * exp(-0.5 * sum_d ((x[n,d] - y[m,d]) / l_d)^2)
with l = exp(log_lengthscale).

Per core (rows of x sharded 8-ways), with invl2[d] = exp(-2*log_lengthscale[d]):
the lengthscale is folded into the X side; the y^2 reduction is folded
into the MAIN matmul by doubling the contraction dim to K=128:
    lhsT = [x*invl2 (64 rows); W (64 rows, W[d,:] = -0.5*invl2[d])]
    rhs  = [y (64 rows);       y^2 (64 rows)]
    psum = cross[n,m] + y2[m],  y2[m] = sum_d -0.5*invl2[d]*y[d,m]^2
    out  = Exp(psum + bias[n]),  bias = -0.5*sum_d x^2*invl2 + log_os (ACT bias)
Matmul cost is N-dependent only, so K=65->128 is free on the PE; this
deletes the per-chunk y2 PSUM borrows and the slow 1-partition DVE y2
row adds of the old scheme. y streams from HBM directly into rhs rows
0:64; one SBUF->SBUF DMA dup per chunk replicates it to rows 64:128
(DVE cannot cross partitions) where one in-place DVE multiply squares
it (f32r-rounding writer). x2 bias rides a single PSUM-borrowed
8-matmul transpose (lhsT=xsq slab, rhs=-0.5*exp(2*lls)) + one DVE add;
bias/x_aug/y_aug are double-buffered across reps so the whole prologue
of rep r+1 overlaps rep r's ACT stream.

exp runs as one ScalarE pass per [128, 2048] PSUM chunk, written as
bf16 (8-bit exponent covers the e^-60-scale tail) and upcast to f32 on
the host — halves the output HBM traffic. ACT busy ~61us/rep is the
roofline; output DMA ~47us + input ~7us rides just under it.

Inputs are staged host-side in transposed layout ([D, points]) so the
contraction dim lands on SBUF partitions with no on-device transposes.

build_nc(repeat=R) emits the whole computation R times into one NEFF
(reps serialized by buffer reuse) — used by test.py to measure the
per-iteration device makespan as a slope, amortizing dispatch overhead.
main_mm/act/outdma=False build ablation variants for differential
timing only (wrong results).
"""

import numpy as np

import concourse.bass as bass
import concourse.mybir as mybir
import concourse.tile as tile
from concourse import bacc
from concourse.bass_utils import run_bass_kernel_spmd

N_CORES = 8
N, M, D = 8192, 8192, 64
NSH = N // N_CORES  # 1024 x-rows per core

F32 = mybir.dt.float32
F32R = mybir.dt.float32r
BF16 = mybir.dt.bfloat16
AF = mybir.ActivationFunctionType


def build_nc(nsh=NSH, m=M, d=D, use_f32r=True, n_chunk=2048, out_bf16=True,
             repeat=1, main_mm=True, act=True, outdma=True, dma_rings=2,
             ot_bufs=6, dup_sq=True, out_pair=False):
    """Per-core Bass graph. SPMD: same graph on all 8 cores."""
    nc = bacc.Bacc("TRN2", target_bir_lowering=False)

    odt = BF16 if out_bf16 else F32

    xT = nc.dram_tensor("xT", [d, nsh], F32, kind="ExternalInput")
    yT = nc.dram_tensor("yT", [d, m], F32, kind="ExternalInput")
    lls = nc.dram_tensor("log_lengthscale", [d], F32, kind="ExternalInput")
    los = nc.dram_tensor("log_outputscale", [1], F32, kind="ExternalInput")
    out = nc.dram_tensor("out", [nsh, m], odt, kind="ExternalOutput")

    n_tiles = nsh // 128          # x tiles (output partition dim)
    mm_n = 512                    # moving free dim per matmul (one PSUM bank)
    n_sub = n_chunk // mm_n       # matmuls per ACT chunk
    mc = m // n_chunk             # y chunks
    K2 = 2 * d                    # contraction: 64 data rows + 64 ysq rows

    def mmi(ap):  # matmul input view
        return ap.bitcast(F32R) if use_f32r else ap

    def mmo(ap):  # rounded-writer output view (BIR fp32r-producer rule)
        return ap.bitcast(F32R) if use_f32r else ap

    # output DMA queue rotation (SP always; optionally ACT HWDGE / SWDGE)
    rings = [nc.sync, nc.scalar, nc.gpsimd][:dma_rings]

    if out_pair:
        # one [128, 2*n_chunk] tile per row tile stays live across two
        # column chunks — fewer bufs than n_tiles+1 would deadlock the
        # ACT FIFO on the pool WAR
        ot_bufs = max(ot_bufs, nsh // 128 + 1)

    with tile.TileContext(nc) as tc:
        with (
            tc.tile_pool(name="const", bufs=1) as cpool,
            tc.tile_pool(name="xr", bufs=2) as xrp,
            tc.tile_pool(name="xq", bufs=2) as xqp,
            tc.tile_pool(name="biasp", bufs=2) as bp,
            tc.tile_pool(name="outp", bufs=ot_bufs) as opool,
            tc.tile_pool(name="mainps", bufs=2, space="PSUM") as mp,
        ):
            # ---- hyperparameters on both partition halves (once) ----
            lls2 = cpool.tile([128, 1], F32, tag="lls2")
            nc.sync.dma_start(out=lls2[0:d, :],
                              in_=lls[:].rearrange("(d o) -> d o", o=1))
            nc.sync.dma_start(out=lls2[d:K2, :],
                              in_=lls[:].rearrange("(d o) -> d o", o=1))
            los_sb = cpool.tile([1, 1], F32, tag="los")
            nc.sync.dma_start(out=los_sb[:, :],
                              in_=los[:].rearrange("(a o) -> a o", o=1))

            invl2 = cpool.tile([128, 1], F32, tag="invl2")  # exp(-2*lls), both
            nc.scalar.activation(invl2[0:K2, :], lls2[0:K2, :], AF.Exp, scale=-2.0)
            l2lo = cpool.tile([128, 1], F32, tag="l2lo")    # exp(+2*lls), low
            nc.scalar.activation(l2lo[0:d, :], lls2[0:d, :], AF.Exp, scale=2.0)
            neghalf = cpool.tile([128, 1], F32, tag="neghalf")
            nc.vector.memset(neghalf[:, :], -0.5)
            # x2 reduce weights -0.5*exp(2*lls) (xsq holds rounded (x*invl2)^2);
            # consumed by the N=1 psb matmuls which must be plain f32 (the
            # fp32r ISA path needs even free counts / 8B-aligned dst)
            nhx2 = cpool.tile([128, 1], F32, tag="nhx2")
            nc.vector.tensor_mul(nhx2[0:d, :], l2lo[0:d, :], neghalf[0:d, :])
            # y2 weights -0.5*invl2 on the UPPER partition half (lhsT W rows)
            nh_hi = cpool.tile([128, 1], F32, tag="nh_hi")
            nc.vector.tensor_mul(mmo(nh_hi[d:K2, :]), invl2[d:K2, :],
                                 neghalf[d:K2, :])

            # log_outputscale broadcast to all 128 partitions via 1-row matmul
            ones1r = cpool.tile([1, 128], F32, tag="ones1r")
            nc.vector.memset(ones1r[:, :], 1.0)
            pstmp = mp.tile([128, n_chunk], F32, tag="mm")
            nc.tensor.matmul(pstmp[:, 0:1], ones1r[:, :], los_sb[:, :],
                             start=True, stop=True)
            los128 = cpool.tile([128, 1], F32, tag="los128")
            nc.vector.tensor_copy(los128[:, :], pstmp[:, 0:1])

            ones_hi = cpool.tile([128, nsh], F32, tag="ones_hi")
            nc.vector.memset(ones_hi[d:K2, :], 1.0)

            # x_aug / y_aug ping-pong across reps; constant W rows written once
            x_augs, y_augs = [], []
            for b in range(2):
                xa = cpool.tile([K2, nsh], F32, tag=f"xaug{b}", name=f"xaug{b}")
                nc.vector.tensor_scalar_mul(mmo(xa[d:K2, :]), ones_hi[d:K2, :],
                                            nh_hi[d:K2, :])
                x_augs.append(xa)
                ya = cpool.tile([K2, m], F32, tag=f"yaug{b}", name=f"yaug{b}")
                y_augs.append(ya)

            for rep in range(repeat):
                x_aug = x_augs[rep % 2]
                y_aug = y_augs[rep % 2]

                # ---- input DMAs on the SWDGE (gpsimd) queue: never queue
                # behind output DMAs on the HWDGE rings ----
                x_raw = xrp.tile([128, nsh], F32, tag="x_raw")
                nc.gpsimd.dma_start(out=x_raw[0:d, :], in_=xT[:, :])
                for jc in range(mc):
                    slc = slice(jc * n_chunk, (jc + 1) * n_chunk)
                    nc.gpsimd.dma_start(out=mmo(y_aug[0:d, slc]),
                                        in_=yT[:, slc].bitcast(F32R)
                                        if use_f32r else yT[:, slc])
                if dup_sq:
                    for jc in range(mc):
                        slc = slice(jc * n_chunk, (jc + 1) * n_chunk)
                        nc.gpsimd.dma_start(out=y_aug[d:K2, slc],
                                            in_=y_aug[0:d, slc])

                def sq(jc):
                    # in-place square of the replicated y rows, f32r writer
                    if not dup_sq:
                        return
                    slc = slice(jc * n_chunk, (jc + 1) * n_chunk)
                    nc.vector.tensor_mul(mmo(y_aug[d:K2, slc]),
                                         y_aug[d:K2, slc], y_aug[d:K2, slc])

                # ---- x side (DVE): x_aug[0:64] = x*invl2, xsq = (x*invl2)^2
                nc.vector.tensor_scalar_mul(mmo(x_aug[0:d, :]), x_raw[0:d, :],
                                            invl2[0:d, :])
                xsq = xqp.tile([128, nsh], F32, tag="xsq")
                nc.vector.tensor_mul(mmo(xsq[0:d, :]), x_aug[0:d, :],
                                     x_aug[0:d, :])

                sq(0)
                sq(1)

                # ---- bias: transposed x2 via one PSUM borrow, then +log_os
                psb = mp.tile([128, n_chunk], F32, tag="mm")
                for j in range(n_tiles):
                    nc.tensor.matmul(
                        psb[:, j : j + 1],
                        xsq[0:d, j * 128 : (j + 1) * 128],
                        nhx2[0:d, 0:1],
                        start=True, stop=True,
                    )
                bias_sb = bp.tile([128, n_tiles], F32, tag="bias")
                nc.vector.tensor_scalar_add(bias_sb[:, :], psb[:, 0:n_tiles],
                                            los128[:, 0:1])

                for jc in range(2, mc):
                    sq(jc)

                # ---- main sweep, column-major: per column chunk, all 8 row
                # tiles matmul+exp+ship. out_pair shares one [128, 2*n_chunk]
                # SBUF tile across two column chunks so outputs ship as 1 MiB
                # DMAs (half the DMA fixed costs) ----
                ot_live = {}
                for c in range(mc):
                    for i in range(n_tiles):
                        ps = mp.tile([128, n_chunk], F32, tag="mm")
                        if main_mm:
                            for jj in range(n_sub):
                                col = c * n_chunk + jj * mm_n
                                nc.tensor.matmul(
                                    ps[:, jj * mm_n : (jj + 1) * mm_n],
                                    mmi(x_aug[:, i * 128 : (i + 1) * 128]),
                                    mmi(y_aug[:, col : col + mm_n]),
                                    start=True, stop=True,
                                )
                        else:
                            nc.vector.memset(ps[:, 0:1], 0.0)
                        if out_pair:
                            if c % 2 == 0:
                                ot = opool.tile([128, 2 * n_chunk], odt, tag="ot")
                                ot_live[i] = ot
                            else:
                                ot = ot_live[i]
                            osl = slice((c % 2) * n_chunk, (c % 2 + 1) * n_chunk)
                        else:
                            ot = opool.tile([128, n_chunk], odt, tag="ot")
                            osl = slice(0, n_chunk)
                        if act:
                            nc.scalar.activation(
                                ot[:, osl], ps[:, :], AF.Exp,
                                bias=bias_sb[:, i : i + 1],
                            )
                        else:
                            nc.vector.memset(ot[:, osl.start : osl.start + 1], 0.0)
                        if outdma and (not out_pair or c % 2 == 1):
                            c0 = (c - 1) if out_pair else c
                            rings[(c * n_tiles + i) % len(rings)].dma_start(
                                out=out[i * 128 : (i + 1) * 128,
                                        c0 * n_chunk : (c + 1) * n_chunk],
                                in_=ot[:, 0 : (c + 1 - c0) * n_chunk],
                            )
    nc.finalize()
    return nc


_NC_CACHE = {}


def _get_nc():
    if "nc" not in _NC_CACHE:
        _NC_CACHE["nc"] = build_nc()
    return _NC_CACHE["nc"]


def stage_inputs(x, y, log_lengthscale, log_outputscale):
    x = np.ascontiguousarray(np.asarray(x, dtype=np.float32))
    y = np.ascontiguousarray(np.asarray(y, dtype=np.float32))
    lls = np.ascontiguousarray(np.asarray(log_lengthscale, dtype=np.float32))
    los = np.ascontiguousarray(np.asarray(log_outputscale, dtype=np.float32))

    yT = np.ascontiguousarray(y.T)  # [D, M]
    in_maps = []
    for c in range(N_CORES):
        xT_c = np.ascontiguousarray(x[c * NSH : (c + 1) * NSH].T)  # [D, NSH]
        in_maps.append(
            {"xT": xT_c, "yT": yT, "log_lengthscale": lls, "log_outputscale": los}
        )
    return in_maps


def assemble_output(out_concat):
    """Map the over-cores-concatenated device output to the final [N, M]."""
    return np.asarray(out_concat).astype(np.float32)


def kernel(x, y, log_lengthscale, log_outputscale):
    in_maps = stage_inputs(x, y, log_lengthscale, log_outputscale)
    res = run_bass_kernel_spmd(_get_nc(), in_maps, core_ids=list(range(N_CORES)))
    return assemble_output(np.concatenate([r["out"] for r in res.results], axis=0))
